# revision 33
# baseline (speedup 1.0000x reference)
"""Trainium2 Bass kernel for the DigitCap forward pass.

Math note: in the reference, C = softmax(sum(A, axis=-2, keepdims=True), axis=-2)
is a softmax over a size-1 axis, so C == 1.0 exactly for any finite input.
The whole attention gram matrix cancels and the computation reduces to

    S[b,m,d] = sum_n (1 + B_prior[m,0,n]) * sum_p W[m,n,d,p] * u[b,n,p]
    out      = squash(S) = (1 - exp(-|S|)) * S / (|S| + 1e-7)

Sharding: M=10 digit caps are covered by 5 cores holding 2 caps each
(uniform SPMD program; the remaining 3 cores run duplicate pairs whose
outputs are discarded). No collectives needed.

Compute per core: contraction over (n,p)=9216 as 9 n-chunks. Each chunk
is ONE wide matmul: lhsT = uT chunk [n=128, (p,b)=128] (stationary),
rhs = W chunk [n=128, (p',m',d)=256] (moving), accumulating into
PSUM[(p,b)=128, (p',m',d)=256]. The p'==p diagonal blocks are the wanted
partial sums; they are extracted and summed over p in the epilogue
(8x streamed compute waste, but the PE is fed 256-wide which it likes).
"""

import os
import numpy as np

B = 16
N = 1152
DP = 8
M = 10
DD = 16
MS = 2           # m-slots per core
NCHUNK = N // 128
EPS = 1e-7

M_PAIRS = [(0, 1), (2, 3), (4, 5), (6, 7), (8, 9), (0, 1), (2, 3), (4, 5)]

_compiled = None


def _build_raw():
    """Raw (non-Tile) build: manual semaphores, no entry/exit barriers."""
    import concourse.bass as bass
    from concourse import bacc, mybir

    nc = bacc.Bacc("TRN2", target_bir_lowering=False, debug=False, num_devices=8)
    f32 = mybir.dt.float32
    f32r = mybir.dt.float32r
    AFT = mybir.ActivationFunctionType

    w_d = nc.dram_tensor("W_s", [MS, N, DD, DP], f32, kind="ExternalInput")
    u_d = nc.dram_tensor("uT", [N, DP, B], f32, kind="ExternalInput")
    bp_d = nc.dram_tensor("BpT", [N, MS], f32, kind="ExternalInput")
    sel_d = nc.dram_tensor("SEL", [128, DP, B], f32, kind="ExternalInput")
    out_d = nc.dram_tensor("out_s", [B, MS, DD], f32, kind="ExternalOutput")

    w_ap = w_d.ap().rearrange("m (c n) d p -> n c m (d p)", n=128)  # [128,9,2,128]
    u_ap = u_d.ap().rearrange("(c n) p b -> n c (p b)", n=128)      # [128,9,128]
    bp_ap = bp_d.ap().rearrange("(c n) m -> n c m", n=128)          # [128,9,2]
    out_ap = out_d.ap()

    NG = 3            # W dma groups
    GSZ = NCHUNK // NG

    from contextlib import ExitStack

    with ExitStack() as ctx:
        sb = lambda name, shape, dt_: ctx.enter_context(
            nc.sbuf_tensor(name, shape, dt_)
        )
        wt = sb("wt", [128, NCHUNK, MS, DD, DP], f32)
        wt_s = sb("wt_s", [128, NCHUNK, MS, DD, DP], f32r)
        ut = sb("ut", [128, NCHUNK, DP, B], f32)
        ut_r = sb("ut_r", [128, NCHUNK, DP, B], f32r)
        cbt = sb("cbt", [128, NCHUNK, MS], f32)
        cb1 = sb("cb1", [128, NCHUNK, MS], f32)
        sel = sb("sel", [128, DP, B], f32)
        ps_sb = sb("ps_sb", [128, DP, MS, DD], f32)
        s = sb("s", [B, MS, DD], f32)
        sq = sb("sq", [B, MS, DD], f32)
        n2 = sb("n2", [B, MS], f32)
        nrm = sb("nrm", [B, MS], f32)
        e_t = sb("e_t", [B, MS], f32)
        coef = sb("coef", [B, MS], f32)
        neps = sb("neps", [B, MS], f32)
        rec = sb("rec", [B, MS], f32)
        fac = sb("fac", [B, MS], f32)
        o = sb("o", [B, MS, DD], f32)
        warm = sb("warm", [B, MS], f32)
        warm2 = sb("warm2", [B, MS], f32)
        ps = ctx.enter_context(nc.psum_tensor("ps", [128, DP, MS, DD], f32))
        ps2 = ctx.enter_context(nc.psum_tensor("ps2", [B, MS, DD], f32))
        sem = lambda name: ctx.enter_context(nc.semaphore(name))
        dcb, du, dsel = sem("dcb"), sem("du"), sem("dsel")
        dw0, dw1, dw2 = sem("dw0"), sem("dw1"), sem("dw2")
        vs, ts, ss, dos = sem("vs"), sem("ts"), sem("ss"), sem("dos")
        dwg = [dw0, dw1, dw2]
        with nc.Block() as block:

            @block.sync
            def _(sync):
                sync.dma_start(cbt[:], bp_ap).then_inc(dcb, 16)
                for g in range(NG):
                    sync.dma_start(
                        wt[:, g * GSZ : (g + 1) * GSZ, 0].rearrange(
                            "n c d p -> n c (d p)"
                        ),
                        w_ap[:, g * GSZ : (g + 1) * GSZ, 0],
                    ).then_inc(dwg[g], 16)
                sync.wait_ge(vs, 38)
                sync.dma_start(out_ap[:], o[:]).then_inc(dos, 16)
                sync.wait_ge(dos, 16)

            @block.scalar
            def _(scalar):
                scalar.dma_start(
                    ut[:].rearrange("n c p b -> n c (p b)"), u_ap
                ).then_inc(du, 16)
                scalar.dma_start(sel[:], sel_d.ap()).then_inc(dsel, 16)
                for g in range(NG):
                    scalar.dma_start(
                        wt[:, g * GSZ : (g + 1) * GSZ, 1].rearrange(
                            "n c d p -> n c (d p)"
                        ),
                        w_ap[:, g * GSZ : (g + 1) * GSZ, 1],
                    ).then_inc(dwg[g], 16)
                # warm the ACT tables (Exp, Sqrt) during the DMA phase
                scalar.wait_ge(dcb, 16)
                nc.scalar.activation(warm[:], cbt[0:B, 0], AFT.Exp, scale=-1.0)
                nc.scalar.activation(
                    warm2[:], cbt[0:B, 0], AFT.Sqrt, bias=1.0, scale=0.0
                )
                # epilogue transcendentals (vector hits vs=32 once n2 is ready)
                scalar.wait_ge(vs, 32)
                nc.scalar.sqrt(nrm[:], n2[:]).then_inc(ss)
                scalar.wait_ge(ss, 1)
                nc.scalar.activation(e_t[:], nrm[:], AFT.Exp, scale=-1.0).then_inc(
                    ss
                )

            @block.vector
            def _(vector):
                vector.wait_ge(dcb, 16)
                nc.vector.tensor_scalar_add(cb1[:], cbt[:], 1.0).then_inc(vs)  # 1
                # tensor_scalar's scalar operand is fetched at dispatch (PTR
                # read) — needs an explicit edge even on the same engine
                vector.wait_ge(vs, 1)
                vector.wait_ge(du, 16)
                for c in range(NCHUNK):
                    nc.vector.tensor_copy(ut_r[:, c], ut[:, c]).then_inc(vs)
                    if c % GSZ == 0:
                        vector.wait_ge(dwg[c // GSZ], 32)
                    nc.vector.tensor_scalar_mul(
                        wt_s[:, c, 0], wt[:, c, 0], cb1[:, c, 0:1]
                    ).then_inc(vs)
                    nc.vector.tensor_scalar_mul(
                        wt_s[:, c, 1], wt[:, c, 1], cb1[:, c, 1:2]
                    ).then_inc(vs)
                # after chunk c: vs = 1 + 3(c+1); final vs = 28
                vector.wait_ge(ts, 1)
                nc.vector.tensor_copy(ps_sb[:], ps[:]).then_inc(vs)  # 29
                vector.wait_ge(ts, 2)
                nc.vector.tensor_copy(s[:], ps2[:]).then_inc(vs)  # 30
                vector.wait_ge(vs, 30)
                nc.vector.tensor_mul(sq[:], s[:], s[:]).then_inc(vs)  # 31
                vector.wait_ge(vs, 31)
                nc.vector.tensor_reduce(
                    n2[:], sq[:], axis=mybir.AxisListType.X, op=mybir.AluOpType.add
                ).then_inc(vs)  # 32
                vector.wait_ge(ss, 2)
                nc.vector.tensor_scalar(
                    coef[:], e_t[:], -1.0, 1.0, mybir.AluOpType.mult,
                    mybir.AluOpType.add,
                ).then_inc(vs)  # 33
                nc.vector.tensor_scalar_add(neps[:], nrm[:], EPS).then_inc(vs)  # 34
                vector.wait_ge(vs, 34)
                nc.vector.reciprocal(rec[:], neps[:]).then_inc(vs)  # 35
                vector.wait_ge(vs, 35)
                nc.vector.tensor_mul(fac[:], coef[:], rec[:]).then_inc(vs)  # 36
                vector.wait_ge(vs, 36)  # fac is a PTR operand below
                nc.vector.tensor_scalar_mul(o[:, 0], s[:, 0], fac[:, 0:1]).then_inc(
                    vs
                )  # 37
                nc.vector.tensor_scalar_mul(o[:, 1], s[:, 1], fac[:, 1:2]).then_inc(
                    vs
                )  # 38

            @block.tensor
            def _(tensor):
                for c in range(NCHUNK):
                    tensor.wait_ge(vs, 1 + 3 * (c + 1))
                    mm = nc.tensor.matmul(
                        ps[:],
                        ut_r[:, c].rearrange("n p b -> n (p b)"),
                        wt_s[:, c].rearrange("n m d p -> n p m d"),
                        start=(c == 0),
                        stop=(c == NCHUNK - 1),
                    )
                    if c == NCHUNK - 1:
                        mm.then_inc(ts)
                tensor.wait_ge(vs, 29)
                tensor.wait_ge(dsel, 16)
                for p in range(DP):
                    mm = nc.tensor.matmul(
                        ps2[:],
                        sel[:, p],
                        ps_sb[:, p],
                        start=(p == 0),
                        stop=(p == DP - 1),
                    )
                    if p == DP - 1:
                        mm.then_inc(ts)

    nc.compile()
    return nc


def _build():
    import concourse.bass as bass
    import concourse.tile as tile
    from concourse import bacc, mybir

    mm_dt = os.environ.get("KERNEL_MM_DT", "f32r")  # f32 | f32r | bf16
    n_wdma = int(os.environ.get("KERNEL_N_WDMA", "3"))  # W dma_start count

    nc = bacc.Bacc("TRN2", target_bir_lowering=False, debug=False, num_devices=8)
    f32 = mybir.dt.float32
    sb_dt = mybir.dt.bfloat16 if mm_dt == "bf16" else f32

    w_d = nc.dram_tensor("W_s", [MS, N, DD, DP], f32, kind="ExternalInput")
    u_d = nc.dram_tensor("uT", [N, DP, B], f32, kind="ExternalInput")
    bp_d = nc.dram_tensor("BpT", [N, MS], f32, kind="ExternalInput")
    sel_d = nc.dram_tensor("SEL", [128, DP, B], f32, kind="ExternalInput")
    out_d = nc.dram_tensor("out_s", [B, MS, DD], f32, kind="ExternalOutput")

    # source views, n-chunked to 128 partitions
    w_ap = w_d.ap().rearrange("m (c n) d p -> n c m (d p)", n=128)     # [128,9,2,128]
    u_ap = u_d.ap().rearrange("(c n) p b -> n c (p b)", n=128)         # [128,9,128]
    bp_ap = bp_d.ap().rearrange("(c n) m -> n c m", n=128)             # [128,9,2]
    out_ap = out_d.ap()

    with tile.TileContext(nc) as tc:
        with (
            tc.tile_pool(name="big", bufs=1) as big,
            tc.tile_pool(name="small", bufs=1) as small,
            tc.tile_pool(name="psum", bufs=1, space="PSUM") as psum,
        ):
            wt = big.tile([128, NCHUNK, MS, DD, DP], sb_dt, tag="wt")
            ut = big.tile([128, NCHUNK, DP, B], sb_dt, tag="ut")
            cbt = small.tile([128, NCHUNK, MS], f32, tag="cbt")
            sel = big.tile([128, DP, B], f32, tag="sel")
            dma_w = nc.gpsimd.dma_start if mm_dt == "bf16" else nc.sync.dma_start
            dma_u = nc.gpsimd.dma_start if mm_dt == "bf16" else nc.scalar.dma_start

            # tiny inputs first so cb1 and the first matmul aren't gated on
            # the big W transfers (HWDGE completion is FIFO per queue lane)
            nc.sync.dma_start(cbt[:], bp_ap)
            dma_u(ut[:].rearrange("n c p b -> n c (p b)"), u_ap)
            nc.scalar.dma_start(sel[:], sel_d.ap())

            # W: split into n_wdma issues so chunk-group g's matmuls can
            # start while group g+1 is still in flight
            assert NCHUNK % n_wdma == 0
            gsz = NCHUNK // n_wdma
            for g in range(n_wdma):
                for ms in range(MS):
                    if mm_dt == "bf16":
                        eng_dma = nc.gpsimd.dma_start
                    else:
                        eng_dma = nc.sync.dma_start if ms == 0 else nc.scalar.dma_start
                    eng_dma(
                        wt[:, g * gsz : (g + 1) * gsz, ms].rearrange(
                            "n c d p -> n c (d p)"
                        ),
                        w_ap[:, g * gsz : (g + 1) * gsz, ms],
                    )

            cb1 = small.tile([128, NCHUNK, MS], f32, tag="cb1")
            nc.vector.tensor_scalar_add(cb1[:], cbt[:], 1.0)

            # hoist ACT table loads (Sqrt/Exp, the only two ACT funcs used) so
            # they overlap the DMA phase instead of stalling the epilogue chain
            warm = small.tile([B, MS], f32, tag="warm")
            nc.scalar.activation(
                warm[:], cb1[0:B, 0], mybir.ActivationFunctionType.Exp, scale=-1.0
            )
            nc.scalar.activation(
                warm[:], warm[:], mybir.ActivationFunctionType.Sqrt
            )

            if mm_dt == "f32r":
                f32r = mybir.dt.float32r
                wt_s = big.tile([128, NCHUNK, MS, DD, DP], f32r, tag="wt_s")
                ut_mm = big.tile([128, NCHUNK, DP, B], f32r, tag="ut_mm")
                nc.vector.tensor_copy(ut_mm[:], ut[:])  # rounds to f32r
            else:
                wt_s = wt
                ut_mm = ut

            ps = psum.tile([128, DP, MS, DD], f32, tag="ps")
            for c in range(NCHUNK):
                # scale W by (1 + B_prior), per (n, chunk, m-slot); for f32r
                # this op also performs the required rounding on its output
                for ms in range(MS):
                    nc.vector.tensor_scalar_mul(
                        wt_s[:, c, ms], wt[:, c, ms], cb1[:, c, ms : ms + 1]
                    )
                nc.tensor.matmul(
                    ps[:],
                    ut_mm[:, c].rearrange("n p b -> n (p b)"),
                    wt_s[:, c].rearrange("n m d p -> n p m d"),
                    start=(c == 0),
                    stop=(c == NCHUNK - 1),
                )

            # diagonal extraction: S[b, m', d] = sum_p ps[16p+b, p, :, :].
            # DVE/walrus reject partition bases that aren't 32-aligned, so the
            # cross-partition gather runs on the PE: out2[b,:] accumulates
            # SEL[:, p].T @ ps_sb[:, p] over p, where SEL[q,p,b] = (q==16p+b).
            f32t = f32
            ps_sb = small.tile([128, DP, MS, DD], f32, tag="ps_sb")
            nc.vector.tensor_copy(ps_sb[:], ps[:])
            ps2 = psum.tile([B, MS, DD], f32, tag="ps2")
            for p in range(DP):
                nc.tensor.matmul(
                    ps2[:],
                    sel[:, p],
                    ps_sb[:, p],
                    start=(p == 0),
                    stop=(p == DP - 1),
                )
            s = small.tile([B, MS, DD], f32t, tag="s")
            nc.vector.tensor_copy(s[:], ps2[:])

            # squash over d per (b, m-slot)
            sq = small.tile([B, MS, DD], f32t, tag="sq")
            nc.vector.tensor_mul(sq[:], s[:], s[:])
            n2 = small.tile([B, MS], f32t, tag="n2")
            nc.vector.tensor_reduce(
                n2[:], sq[:], axis=mybir.AxisListType.X, op=mybir.AluOpType.add
            )
            nrm = small.tile([B, MS], f32t, tag="nrm")
            nc.scalar.sqrt(nrm[:], n2[:])
            e = small.tile([B, MS], f32t, tag="e")
            nc.scalar.activation(
                e[:], nrm[:], mybir.ActivationFunctionType.Exp, scale=-1.0
            )
            coef = small.tile([B, MS], f32t, tag="coef")
            nc.vector.tensor_scalar(
                coef[:], e[:], -1.0, 1.0, mybir.AluOpType.mult, mybir.AluOpType.add
            )
            neps = small.tile([B, MS], f32t, tag="neps")
            nc.vector.tensor_scalar_add(neps[:], nrm[:], EPS)
            rec = small.tile([B, MS], f32t, tag="rec")
            nc.vector.reciprocal(rec[:], neps[:])
            fac = small.tile([B, MS], f32t, tag="fac")
            nc.vector.tensor_mul(fac[:], coef[:], rec[:])
            o = small.tile([B, MS, DD], f32, tag="o")
            for ms in range(MS):
                nc.vector.tensor_scalar_mul(o[:, ms], s[:, ms], fac[:, ms : ms + 1])
            nc.sync.dma_start(out_ap[:], o[:])

    nc.compile()
    return nc


def make_in_maps(primary_caps: np.ndarray, W: np.ndarray, B_prior: np.ndarray):
    u = np.ascontiguousarray(primary_caps, dtype=np.float32)
    uT = np.ascontiguousarray(u.transpose(1, 2, 0))  # [N, DP, B]
    sel = np.zeros((128, DP, B), dtype=np.float32)
    for p in range(DP):
        for b in range(B):
            sel[16 * p + b, p, b] = 1.0
    in_maps = []
    for pr in M_PAIRS:
        in_maps.append(
            {
                "W_s": np.ascontiguousarray(W[list(pr)], dtype=np.float32),
                "uT": uT,
                "BpT": np.ascontiguousarray(
                    B_prior[list(pr), 0, :].T.astype(np.float32)
                ),
                "SEL": sel,
            }
        )
    return in_maps


def kernel(primary_caps: np.ndarray, W: np.ndarray, B_prior: np.ndarray) -> np.ndarray:
    from concourse.bass_utils import run_bass_kernel_spmd

    global _compiled
    if _compiled is None:
        if os.environ.get("KERNEL_IMPL", "raw") == "raw":
            _compiled = _build_raw()
        else:
            _compiled = _build()
    nc = _compiled

    in_maps = make_in_maps(primary_caps, W, B_prior)
    res = run_bass_kernel_spmd(nc, in_maps, list(range(8))).results
    out = np.empty((B, M, DD), dtype=np.float32)
    for i in range(5):
        out[:, 2 * i : 2 * i + 2, :] = res[i]["out_s"]
    return out
